# revision 19
# baseline (speedup 1.0000x reference)
"""Causal self-attention layer (B=4, T=2048, C=1024, H=16) on 8 TRN2 NeuronCores.

Sharding: Megatron-style tensor parallel over heads — 2 heads per core.
Each core computes q/k/v projections for its 2 heads, causal attention with an
appended-ones column on V for softmax denominators, and a partial output
projection against its 128-row slice of W_proj. The host sums the 8 partial
projections and adds b_proj.

v2 changes vs the f32r baseline (652 us):
- All matmul operands bf16, converted on the HOST (x, weights, emat), so the
  device never casts x (removes ~68 us of DVE CAST work and halves input DMA).
- Scores for two k-tiles land in one 2-bank [128,1024] PSUM tile so each ACT
  exp covers both (ACT was the attention pacing engine at 687 ns/512-tile).
- DMA spread across the three DMA-capable issue engines (sync/scalar HWDGE +
  gpsimd SWDGE) instead of serializing everything on the sync queue.
- V transposed 128 rows at a time (both heads per instruction) into a resident
  [128, kt, 130] tile with two constant ones columns (written once).
- Denominator reciprocal via the single-instruction reciprocal_approx_fast
  (the exact DVE reciprocal measured 3.3 us per batch).
- Normalization + output projection of batch b deferred until after attention
  of batch b+1 is emitted, hiding the reciprocal chain and output DMA.
- Output partials in bf16 (halves output DMA; host sums partials in f32).
"""
import sys

sys.path.insert(0, "/opt/trn_rl_repo")

import ml_dtypes
import numpy as np

import concourse.bass as bass  # noqa: F401
from concourse import bacc
import concourse.mybir as mybir
import concourse.tile as tile
from concourse.bass_utils import run_bass_kernel_spmd
from concourse.masks import make_identity

B, T, C = 4, 2048, 1024
H, DH = 16, 64
N_CORES = 8
HPC = H // N_CORES          # heads per core = 2
DPC = HPC * DH              # head-dims per core = 128
NT = B * T                  # 8192 tokens
CH = C // 128               # 8 contraction chunks
QB = 512                    # q-block width (moving dim)
KT = 128                    # k-tile width (PE partition dim)
SCALE = 1.0 / 8.0           # 1/sqrt(DH)

F32 = mybir.dt.float32
BF16 = mybir.dt.bfloat16
AF = mybir.ActivationFunctionType
NPBF = ml_dtypes.bfloat16

_CACHED_NC = None
LAST_RESULT = None


def _build():
    nc = bacc.Bacc(None)

    xT = nc.dram_tensor("xT", [C, NT], BF16, kind="ExternalInput")
    wq = nc.dram_tensor("wq", [C, DPC], BF16, kind="ExternalInput")
    wk = nc.dram_tensor("wk", [C, DPC], BF16, kind="ExternalInput")
    wv = nc.dram_tensor("wv", [C, DPC], BF16, kind="ExternalInput")
    bq = nc.dram_tensor("bq", [DPC, 1], F32, kind="ExternalInput")
    bk = nc.dram_tensor("bk", [DPC, 1], F32, kind="ExternalInput")
    bv = nc.dram_tensor("bv", [DPC, 1], F32, kind="ExternalInput")
    wp = nc.dram_tensor("wp", [DPC, C], BF16, kind="ExternalInput")
    emat_in = nc.dram_tensor("emat", [8, 4, 128], BF16, kind="ExternalInput")
    out = nc.dram_tensor("out", [NT, C], BF16, kind="ExternalOutput")

    with tile.TileContext(nc) as tc:
        with (
            tc.tile_pool(name="const", bufs=1) as const,
            tc.tile_pool(name="res", bufs=1) as res,
        ):
            # --- constants ---
            ident = const.tile([128, 128], BF16, tag="ident")
            # sliding causal mask: wmask[k, u] = 1 iff k <= u - 512; a crossing
            # tile r multiplies by wmask[:, 512-128r : 1024-128r]
            wmask = const.tile([128, 1024], BF16, tag="wmask")
            emat = const.tile([8, 4, 128], BF16, tag="emat")
            with tc.tile_pool(name="cstage", bufs=1) as cstage:
                ident_s = cstage.tile([128, 128], F32, tag="ident_s")
                make_identity(nc, ident_s[:])
                nc.vector.tensor_copy(ident[:], ident_s[:])

                wmask_s = cstage.tile([128, 1024], F32, tag="wmask_s")
                nc.gpsimd.memset(wmask_s[:], 0.0)
                nc.gpsimd.affine_select(
                    out=wmask_s[:],
                    in_=wmask_s[:],
                    compare_op=mybir.AluOpType.is_gt,
                    fill=1.0,
                    base=512,
                    # keep 0 where (512 + k - u) > 0, fill 1 where k <= u - 512
                    pattern=[[-1, 1024]],
                    channel_multiplier=1,
                )
                nc.vector.tensor_copy(wmask[:], wmask_s[:])

                nc.sync.dma_start(emat[:], emat_in[:])

            bq_t = const.tile([DPC, 1], F32, tag="bq")
            bk_t = const.tile([DPC, 1], F32, tag="bk")
            bv_t = const.tile([DPC, 1], F32, tag="bv")
            nc.sync.dma_start(bq_t[:], bq[:])
            nc.sync.dma_start(bk_t[:], bk[:])
            nc.sync.dma_start(bv_t[:], bv[:])

            # weights straight into SBUF as bf16 (host pre-converted)
            wq_r = const.tile([128, CH, DPC], BF16, tag="wq_r")
            wk_r = const.tile([128, CH, DPC], BF16, tag="wk_r")
            wv_r = const.tile([128, CH, DPC], BF16, tag="wv_r")
            wp_r = const.tile([DPC, C], BF16, tag="wp_r")
            for w_in, w_dst in ((wq, wq_r), (wk, wk_r), (wv, wv_r)):
                w_re = w_in.rearrange("(c p) n -> p c n", p=128)
                nc.scalar.dma_start(w_dst[:], w_re)
            nc.sync.dma_start(wp_r[:], wp[:])

            # --- residents ---
            qT = res.tile([DPC, NT], BF16, tag="qT")
            kT = res.tile([DPC, NT], BF16, tag="kT")
            vT = res.tile([DPC, NT], BF16, tag="vT")
            yT = res.tile([DPC, NT], BF16, tag="yT")
            # per-batch transposed V (double-buffered so batch b's DMA
            # transposes overlap batch b-1's attention):
            # [tok128, buf, kt, 0:64]=h0 dims, 64=ones, [65:129]=h1, 129=ones
            v_res = res.tile([128, 2, T // KT, 130], BF16, tag="v_res")
            nc.gpsimd.memset(v_res[:, :, :, 64:65], 1.0)
            nc.gpsimd.memset(v_res[:, :, :, 129:130], 1.0)

            # ================= phase 1: q/k/v projections =================
            xT_re = xT.rearrange("(c p) t -> c p t", p=128)
            dma_engines = [nc.sync, nc.scalar, nc.gpsimd]
            with (
                tc.tile_pool(name="xpool", bufs=12) as xpool,
                tc.tile_pool(name="qkv_psum", bufs=2, space="PSUM") as qkv_psum,
            ):
                for tt in range(NT // QB):
                    ts_ = slice(tt * QB, (tt + 1) * QB)
                    xrs = []
                    for c in range(CH):
                        xs = xpool.tile([128, QB], BF16, tag="xs")
                        eng = dma_engines[(tt * CH + c) % 3]
                        eng.dma_start(xs[:], xT_re[c, :, ts_])
                        xrs.append(xs)
                    psq = qkv_psum.tile([128, QB], F32, tag="psq")
                    psk = qkv_psum.tile([128, QB], F32, tag="psk")
                    psv = qkv_psum.tile([128, QB], F32, tag="psv")
                    for ps, w_r in ((psq, wq_r), (psk, wk_r), (psv, wv_r)):
                        for c in range(CH):
                            nc.tensor.matmul(
                                ps[:], w_r[:, c, :], xrs[c][:],
                                start=(c == 0), stop=(c == CH - 1),
                            )
                    # copy out of PSUM (+bias; q also scaled by 1/sqrt(dh))
                    nc.scalar.activation(qT[:, ts_], psq[:], AF.Identity, bias=bq_t[:], scale=SCALE)
                    nc.scalar.activation(kT[:, ts_], psk[:], AF.Identity, bias=bk_t[:])
                    nc.vector.tensor_scalar_add(vT[:, ts_], psv[:], bv_t[:])

            # ============ phase 2+3: attention + proj, per batch ==========
            with (
                tc.tile_pool(name="epool", bufs=6) as epool,
                tc.tile_pool(name="dpool", bufs=2) as dpool,
                tc.tile_pool(name="opool", bufs=4) as opool,
                tc.tile_pool(name="s_psum", bufs=2, space="PSUM") as s_psum,
                tc.tile_pool(name="y_psum", bufs=2, space="PSUM") as y_psum,
                tc.tile_pool(name="m_psum", bufs=1, space="PSUM") as m_psum,
                tc.tile_pool(name="p_psum", bufs=1, space="PSUM") as p_psum,
            ):
                n_ktiles = T // KT  # 16

                def emit_v_transpose(b, kt):
                    """PE-transpose one 128-token V tile (both heads at once)
                    for batch b into v_res[:, b%2]."""
                    cb = b * T
                    pt = m_psum.tile([128, 128], BF16, tag="mps", name=f"pt{b}_{kt}")
                    nc.tensor.transpose(
                        pt[:], vT[:, cb + kt * KT : cb + (kt + 1) * KT], ident[:]
                    )
                    nc.vector.tensor_copy(v_res[:, b % 2, kt, 0:64], pt[:, 0:64])
                    nc.vector.tensor_copy(v_res[:, b % 2, kt, 65:129], pt[:, 64:128])

                def emit_norm(b, rec_bf):
                    cb = b * T
                    for qb in range(T // QB):
                        qs = slice(cb + qb * QB, cb + (qb + 1) * QB)
                        pb = s_psum.tile([128, 1024], F32, tag="ps", name=f"pb{b}_{qb}")
                        nc.tensor.matmul(
                            pb[:, :512], emat[:, qb, :], rec_bf[:],
                            start=True, stop=True,
                        )
                        nc.vector.tensor_mul(yT[:, qs], yT[:, qs], pb[:, :512])

                def emit_proj_unit(b, u, alt_pool=False):
                    """One [128 tok, 512 col] half-tile of batch b's partial
                    projection: single matmul + evac + DMA. With alt_pool
                    (the tail, when attention PSUM is idle) rotate across the
                    other pools for depth instead of the single pp slot."""
                    tt, half = u // 2, u % 2
                    trow = b * T + tt * 128
                    if not alt_pool or u % 4 == 0:
                        pp = p_psum.tile([128, 512], F32, tag="pp", name=f"pp{b}_{u}")[:]
                    elif u % 2 == 1:
                        pp = s_psum.tile([128, 1024], F32, tag="ps", name=f"pp{b}_{u}")[:, :512]
                    else:
                        pp = y_psum.tile([128, 512], F32, tag="py", name=f"pp{b}_{u}")[:]
                    nc.tensor.matmul(
                        pp,
                        yT[:, trow : trow + 128],
                        wp_r[:, half * 512 : (half + 1) * 512],
                        start=True, stop=True,
                    )
                    os_ = opool.tile([128, 512], BF16, tag="os", name=f"os{b}_{u}")
                    # alternate evac engine: ACT is exp-loaded during the
                    # overlapped attention, DVE has more slack
                    if u % 2 == 0:
                        nc.vector.tensor_copy(os_[:], pp)
                    else:
                        nc.scalar.copy(os_[:], pp)
                    eng = nc.sync if u % 2 == 0 else nc.gpsimd
                    eng.dma_start(
                        out[trow : trow + 128, half * 512 : (half + 1) * 512],
                        os_[:],
                    )

                def emit_attention(b, prev, rec_prev):
                    """Scores+exp+PV for batch b. Batch prev's norm is
                    emitted after the first q-block (its reciprocal chain has
                    drained by then); prev's 32 projection half-tiles and
                    batch b+1's V transposes are interleaved one per score
                    pair so the depth-1 proj PSUM never stalls the PE
                    (attention is ACT-exp paced, PE has slack per pair)."""
                    cb = b * T
                    denw = dpool.tile([1, 8 * QB], F32, tag="denw", name=f"denw{b}")
                    den = dpool.tile([8, QB], F32, tag="den", name=f"den{b}")
                    proj_u = 0
                    vt_kt = 0
                    blocks = 0
                    for hl in range(HPC):
                        rb = hl * DH
                        vlo, vhi = (0, 65) if hl == 0 else (65, 130)
                        for qb in range(T // QB):
                            qs = slice(cb + qb * QB, cb + (qb + 1) * QB)
                            py = y_psum.tile([128, QB], F32, tag="py", name=f"py{b}_{hl}_{qb}")
                            nkt = (qb + 1) * (QB // KT)
                            for p0 in range(nkt // 2):
                                ps2 = s_psum.tile(
                                    [128, 1024], F32, tag="ps",
                                    name=f"ps{b}_{hl}_{qb}_{p0}",
                                )
                                for j in range(2):
                                    kt = 2 * p0 + j
                                    nc.tensor.matmul(
                                        ps2[:, j * 512 : (j + 1) * 512],
                                        kT[rb : rb + DH, cb + kt * KT : cb + (kt + 1) * KT],
                                        qT[rb : rb + DH, qs],
                                        start=True, stop=True,
                                    )
                                ex2 = epool.tile([128, 1024], BF16, tag="ex", name=f"ex{b}_{hl}_{qb}_{p0}")
                                nc.scalar.activation(ex2[:], ps2[:], AF.Exp)
                                for j in range(2):
                                    kt = 2 * p0 + j
                                    r = kt - qb * (QB // KT)
                                    if r >= 0:
                                        # diagonal-crossing tile: zero k > q
                                        # (gpsimd — DVE carries den/yT copies)
                                        nc.gpsimd.tensor_mul(
                                            ex2[:, j * 512 : (j + 1) * 512],
                                            ex2[:, j * 512 : (j + 1) * 512],
                                            wmask[:, 512 - r * KT : 1024 - r * KT],
                                        )
                                for j in range(2):
                                    kt = 2 * p0 + j
                                    nc.tensor.matmul(
                                        py[:65],
                                        v_res[:, b % 2, kt, vlo:vhi],
                                        ex2[:, j * 512 : (j + 1) * 512],
                                        start=(kt == 0), stop=(kt == nkt - 1),
                                    )
                                # interleave one proj half-tile of prev batch
                                # or one V transpose of the next batch
                                if blocks >= 1 and prev is not None and proj_u < 32:
                                    emit_proj_unit(prev, proj_u)
                                    proj_u += 1
                                elif b + 1 < B and vt_kt < n_ktiles:
                                    emit_v_transpose(b + 1, vt_kt)
                                    vt_kt += 1
                            p = hl * 4 + qb
                            nc.vector.tensor_copy(
                                denw[:, p * QB : (p + 1) * QB], py[64:65, :]
                            )
                            nc.vector.tensor_copy(yT[rb : rb + DH, qs], py[:DH, :])
                            blocks += 1
                            if blocks == 1 and prev is not None:
                                emit_norm(prev, rec_prev)
                    while b + 1 < B and vt_kt < n_ktiles:
                        emit_v_transpose(b + 1, vt_kt)
                        vt_kt += 1
                    # scatter den rows to 8 partitions (compute engines can
                    # only write partition bases 0/32/64/96)
                    for p in range(8):
                        nc.sync.dma_start(
                            den[p : p + 1, :], denw[:, p * QB : (p + 1) * QB]
                        )
                    rec = dpool.tile([8, QB], F32, tag="rec", name=f"rec{b}")
                    rec_bf = dpool.tile([8, QB], BF16, tag="rec_bf", name=f"recb{b}")
                    with nc.allow_low_precision(reason="softmax denom recip"):
                        nc.vector.reciprocal_approx_fast(rec[:], den[:])
                        nc.vector.tensor_copy(rec_bf[:], rec[:])
                    return rec_bf

                # batch 0's V transposes up front; later batches' transposes
                # and norm/proj of b-1 ride inside attention(b)
                for kt in range(n_ktiles):
                    emit_v_transpose(0, kt)
                pending = None  # rec_bf of the batch awaiting norm+proj
                for b in range(B):
                    rec_bf = emit_attention(
                        b, b - 1 if pending is not None else None, pending
                    )
                    pending = rec_bf
                emit_norm(B - 1, pending)
                for u in range(32):
                    emit_proj_unit(B - 1, u, alt_pool=True)

    nc.compile()
    return nc


def _get_nc():
    global _CACHED_NC
    if _CACHED_NC is None:
        _CACHED_NC = _build()
    return _CACHED_NC


def kernel(x, W_qkv, b_qkv, W_proj, b_proj, _trace=False, _core_ids=None):
    global LAST_RESULT
    x = np.asarray(x, dtype=np.float32)
    W_qkv = np.asarray(W_qkv, dtype=np.float32)
    b_qkv = np.asarray(b_qkv, dtype=np.float32)
    W_proj = np.asarray(W_proj, dtype=np.float32)
    b_proj = np.asarray(b_proj, dtype=np.float32)

    xT = np.ascontiguousarray(x.reshape(NT, C).T).astype(NPBF)
    emat_np = np.zeros((8, 4, 128), dtype=NPBF)
    for qb in range(4):
        emat_np[qb, qb, :DH] = 1.0
        emat_np[4 + qb, qb, DH:] = 1.0
    core_ids = list(range(N_CORES)) if _core_ids is None else _core_ids
    in_maps = []
    for core in range(len(core_ids)):
        s = slice(core * DPC, (core + 1) * DPC)
        in_maps.append(
            {
                "xT": xT,
                "wq": np.ascontiguousarray(W_qkv[:, 0 * C + core * DPC : 0 * C + (core + 1) * DPC]).astype(NPBF),
                "wk": np.ascontiguousarray(W_qkv[:, 1 * C + core * DPC : 1 * C + (core + 1) * DPC]).astype(NPBF),
                "wv": np.ascontiguousarray(W_qkv[:, 2 * C + core * DPC : 2 * C + (core + 1) * DPC]).astype(NPBF),
                # device computes qT = psq*SCALE + bias, so pre-scale the q bias
                "bq": np.ascontiguousarray(b_qkv[0 * C + core * DPC : 0 * C + (core + 1) * DPC, None]) * np.float32(SCALE),
                "bk": np.ascontiguousarray(b_qkv[1 * C + core * DPC : 1 * C + (core + 1) * DPC, None]),
                "bv": np.ascontiguousarray(b_qkv[2 * C + core * DPC : 2 * C + (core + 1) * DPC, None]),
                "wp": np.ascontiguousarray(W_proj[s, :]).astype(NPBF),
                "emat": emat_np,
            }
        )

    nc = _get_nc()
    res = run_bass_kernel_spmd(nc, in_maps, core_ids, trace=_trace)
    LAST_RESULT = res

    acc = np.zeros((NT, C), dtype=np.float32)
    for r in res.results:
        acc += r["out"].astype(np.float32)
    acc += b_proj
    return acc.reshape(B, T, C).astype(np.float32)


# revision 20
# speedup vs baseline: 1.1781x; 1.1781x over previous
"""Causal self-attention layer (B=4, T=2048, C=1024, H=16) on 8 TRN2 NeuronCores.

Sharding: Megatron-style tensor parallel over heads — 2 heads per core.
Each core computes q/k/v projections for its 2 heads, causal attention with an
appended-ones column on V for softmax denominators, and a partial output
projection against its 128-row slice of W_proj. The host sums the 8 partial
projections and adds b_proj.

v2 changes vs the f32r baseline (652 us):
- All matmul operands bf16, converted on the HOST (x, weights, emat), so the
  device never casts x (removes ~68 us of DVE CAST work and halves input DMA).
- Scores for two k-tiles land in one 2-bank [128,1024] PSUM tile so each ACT
  exp covers both (ACT was the attention pacing engine at 687 ns/512-tile).
- DMA spread across the three DMA-capable issue engines (sync/scalar HWDGE +
  gpsimd SWDGE) instead of serializing everything on the sync queue.
- V transposed 128 rows at a time (both heads per instruction) into a resident
  [128, kt, 130] tile with two constant ones columns (written once).
- Denominator reciprocal via the single-instruction reciprocal_approx_fast
  (the exact DVE reciprocal measured 3.3 us per batch).
- Normalization + output projection of batch b deferred until after attention
  of batch b+1 is emitted, hiding the reciprocal chain and output DMA.
- Output partials in bf16 (halves output DMA; host sums partials in f32).
"""
import sys

sys.path.insert(0, "/opt/trn_rl_repo")

import ml_dtypes
import numpy as np

import concourse.bass as bass  # noqa: F401
from concourse import bacc
import concourse.mybir as mybir
import concourse.tile as tile
from concourse.bass_utils import run_bass_kernel_spmd
from concourse.masks import make_identity

B, T, C = 4, 2048, 1024
H, DH = 16, 64
N_CORES = 8
HPC = H // N_CORES          # heads per core = 2
DPC = HPC * DH              # head-dims per core = 128
NT = B * T                  # 8192 tokens
CH = C // 128               # 8 contraction chunks
QB = 512                    # q-block width (moving dim)
KT = 128                    # k-tile width (PE partition dim)
SCALE = 1.0 / 8.0           # 1/sqrt(DH)

F32 = mybir.dt.float32
BF16 = mybir.dt.bfloat16
AF = mybir.ActivationFunctionType
NPBF = ml_dtypes.bfloat16

_CACHED_NC = None
LAST_RESULT = None


def _build():
    nc = bacc.Bacc(None)

    xT = nc.dram_tensor("xT", [C, NT], BF16, kind="ExternalInput")
    wq = nc.dram_tensor("wq", [C, DPC], BF16, kind="ExternalInput")
    wk = nc.dram_tensor("wk", [C, DPC], BF16, kind="ExternalInput")
    wv = nc.dram_tensor("wv", [C, DPC], BF16, kind="ExternalInput")
    bq = nc.dram_tensor("bq", [DPC, 1], F32, kind="ExternalInput")
    bk = nc.dram_tensor("bk", [DPC, 1], F32, kind="ExternalInput")
    bv = nc.dram_tensor("bv", [DPC, 1], F32, kind="ExternalInput")
    wp = nc.dram_tensor("wp", [DPC, C], BF16, kind="ExternalInput")
    emat_in = nc.dram_tensor("emat", [8, 4, 128], BF16, kind="ExternalInput")
    out = nc.dram_tensor("out", [NT, C], BF16, kind="ExternalOutput")

    with tile.TileContext(nc) as tc:
        with (
            tc.tile_pool(name="const", bufs=1) as const,
            tc.tile_pool(name="res", bufs=1) as res,
        ):
            # --- constants ---
            ident = const.tile([128, 128], BF16, tag="ident")
            # sliding causal mask: wmask[k, u] = 1 iff k <= u - 512; a crossing
            # tile r multiplies by wmask[:, 512-128r : 1024-128r]
            wmask = const.tile([128, 1024], BF16, tag="wmask")
            emat = const.tile([8, 4, 128], BF16, tag="emat")
            with tc.tile_pool(name="cstage", bufs=1) as cstage:
                ident_s = cstage.tile([128, 128], F32, tag="ident_s")
                make_identity(nc, ident_s[:])
                nc.vector.tensor_copy(ident[:], ident_s[:])

                wmask_s = cstage.tile([128, 1024], F32, tag="wmask_s")
                nc.gpsimd.memset(wmask_s[:], 0.0)
                nc.gpsimd.affine_select(
                    out=wmask_s[:],
                    in_=wmask_s[:],
                    compare_op=mybir.AluOpType.is_gt,
                    fill=1.0,
                    base=512,
                    # keep 0 where (512 + k - u) > 0, fill 1 where k <= u - 512
                    pattern=[[-1, 1024]],
                    channel_multiplier=1,
                )
                nc.vector.tensor_copy(wmask[:], wmask_s[:])

                nc.sync.dma_start(emat[:], emat_in[:])

            bq_t = const.tile([DPC, 1], F32, tag="bq")
            bk_t = const.tile([DPC, 1], F32, tag="bk")
            bv_t = const.tile([DPC, 1], F32, tag="bv")
            nc.sync.dma_start(bq_t[:], bq[:])
            nc.sync.dma_start(bk_t[:], bk[:])
            nc.sync.dma_start(bv_t[:], bv[:])

            # weights straight into SBUF as bf16 (host pre-converted)
            wq_r = const.tile([128, CH, DPC], BF16, tag="wq_r")
            wk_r = const.tile([128, CH, DPC], BF16, tag="wk_r")
            wv_r = const.tile([128, CH, DPC], BF16, tag="wv_r")
            wp_r = const.tile([DPC, C], BF16, tag="wp_r")
            for w_in, w_dst in ((wq, wq_r), (wk, wk_r), (wv, wv_r)):
                w_re = w_in.rearrange("(c p) n -> p c n", p=128)
                nc.scalar.dma_start(w_dst[:], w_re)
            nc.sync.dma_start(wp_r[:], wp[:])

            # --- residents ---
            qT = res.tile([DPC, NT], BF16, tag="qT")
            kT = res.tile([DPC, NT], BF16, tag="kT")
            vT = res.tile([DPC, NT], BF16, tag="vT")
            yT = res.tile([DPC, NT], BF16, tag="yT")
            # per-batch transposed V (double-buffered so batch b's DMA
            # transposes overlap batch b-1's attention):
            # [tok128, buf, kt, 0:64]=h0 dims, 64=ones, [65:129]=h1, 129=ones
            v_res = res.tile([128, 2, T // KT, 130], BF16, tag="v_res")
            nc.gpsimd.memset(v_res[:, :, :, 64:65], 1.0)
            nc.gpsimd.memset(v_res[:, :, :, 129:130], 1.0)

            # ================= phase 1: q/k/v projections =================
            xT_re = xT.rearrange("(c p) t -> c p t", p=128)
            dma_engines = [nc.sync, nc.scalar, nc.gpsimd]
            with (
                tc.tile_pool(name="xpool", bufs=12) as xpool,
                tc.tile_pool(name="qkv_psum", bufs=2, space="PSUM") as qkv_psum,
            ):
                for tt in range(NT // QB):
                    ts_ = slice(tt * QB, (tt + 1) * QB)
                    xrs = []
                    for c in range(CH):
                        xs = xpool.tile([128, QB], BF16, tag="xs")
                        eng = dma_engines[(tt * CH + c) % 3]
                        eng.dma_start(xs[:], xT_re[c, :, ts_])
                        xrs.append(xs)
                    psq = qkv_psum.tile([128, QB], F32, tag="psq")
                    psk = qkv_psum.tile([128, QB], F32, tag="psk")
                    psv = qkv_psum.tile([128, QB], F32, tag="psv")
                    for ps, w_r in ((psq, wq_r), (psk, wk_r), (psv, wv_r)):
                        for c in range(CH):
                            nc.tensor.matmul(
                                ps[:], w_r[:, c, :], xrs[c][:],
                                start=(c == 0), stop=(c == CH - 1),
                            )
                    # copy out of PSUM (+bias; q also scaled by 1/sqrt(dh))
                    nc.scalar.activation(qT[:, ts_], psq[:], AF.Identity, bias=bq_t[:], scale=SCALE)
                    nc.scalar.activation(kT[:, ts_], psk[:], AF.Identity, bias=bk_t[:])
                    nc.vector.tensor_scalar_add(vT[:, ts_], psv[:], bv_t[:])

            # ============ phase 2+3: attention + proj, per batch ==========
            with (
                tc.tile_pool(name="epool", bufs=6) as epool,
                tc.tile_pool(name="dpool", bufs=2) as dpool,
                tc.tile_pool(name="opool", bufs=4) as opool,
                tc.tile_pool(name="s_psum", bufs=2, space="PSUM") as s_psum,
                tc.tile_pool(name="y_psum", bufs=2, space="PSUM") as y_psum,
                tc.tile_pool(name="m_psum", bufs=1, space="PSUM") as m_psum,
                tc.tile_pool(name="p_psum", bufs=1, space="PSUM") as p_psum,
            ):
                n_ktiles = T // KT  # 16

                def emit_v_transpose(b, kt):
                    """PE-transpose one 128-token V tile (both heads at once)
                    for batch b into v_res[:, b%2]."""
                    cb = b * T
                    pt = m_psum.tile([128, 128], BF16, tag="mps", name=f"pt{b}_{kt}")
                    nc.tensor.transpose(
                        pt[:], vT[:, cb + kt * KT : cb + (kt + 1) * KT], ident[:]
                    )
                    nc.vector.tensor_copy(v_res[:, b % 2, kt, 0:64], pt[:, 0:64])
                    nc.vector.tensor_copy(v_res[:, b % 2, kt, 65:129], pt[:, 64:128])

                def emit_norm(b, rec_bf):
                    cb = b * T
                    for qb in range(T // QB):
                        qs = slice(cb + qb * QB, cb + (qb + 1) * QB)
                        pb = s_psum.tile([128, 1024], F32, tag="ps", name=f"pb{b}_{qb}")
                        nc.tensor.matmul(
                            pb[:, :512], emat[:, qb, :], rec_bf[:],
                            start=True, stop=True,
                        )
                        nc.vector.tensor_mul(yT[:, qs], yT[:, qs], pb[:, :512])

                def emit_proj_unit(b, u, alt_pool=False):
                    """One [128 tok, 512 col] half-tile of batch b's partial
                    projection: single matmul + evac + DMA. With alt_pool
                    (the tail, when attention PSUM is idle) rotate across the
                    other pools for depth instead of the single pp slot."""
                    tt, half = u // 2, u % 2
                    trow = b * T + tt * 128
                    if not alt_pool or u % 4 == 0:
                        pp = p_psum.tile([128, 512], F32, tag="pp", name=f"pp{b}_{u}")[:]
                    elif u % 2 == 1:
                        pp = s_psum.tile([128, 1024], F32, tag="ps", name=f"pp{b}_{u}")[:, :512]
                    else:
                        pp = y_psum.tile([128, 512], F32, tag="py", name=f"pp{b}_{u}")[:]
                    nc.tensor.matmul(
                        pp,
                        yT[:, trow : trow + 128],
                        wp_r[:, half * 512 : (half + 1) * 512],
                        start=True, stop=True,
                    )
                    os_ = opool.tile([128, 512], BF16, tag="os", name=f"os{b}_{u}")
                    # alternate evac engine: ACT is exp-loaded during the
                    # overlapped attention, DVE has more slack
                    if u % 2 == 0:
                        nc.vector.tensor_copy(os_[:], pp)
                    else:
                        nc.scalar.copy(os_[:], pp)
                    eng = nc.sync if u % 2 == 0 else nc.gpsimd
                    eng.dma_start(
                        out[trow : trow + 128, half * 512 : (half + 1) * 512],
                        os_[:],
                    )

                def emit_attention(b, prev, rec_prev):
                    """Scores+exp+PV for batch b. Batch prev's norm is
                    emitted after the first q-block (its reciprocal chain has
                    drained by then); prev's 32 projection half-tiles and
                    batch b+1's V transposes are interleaved one per score
                    pair so the depth-1 proj PSUM never stalls the PE
                    (attention is ACT-exp paced, PE has slack per pair)."""
                    cb = b * T
                    denw = dpool.tile([1, 8 * QB], F32, tag="denw", name=f"denw{b}")
                    den = dpool.tile([8, QB], F32, tag="den", name=f"den{b}")
                    proj_u = 0
                    vt_kt = 0
                    blocks = 0
                    for hl in range(HPC):
                        rb = hl * DH
                        vlo, vhi = (0, 65) if hl == 0 else (65, 130)
                        for qb in range(T // QB):
                            qs = slice(cb + qb * QB, cb + (qb + 1) * QB)
                            py = y_psum.tile([128, QB], F32, tag="py", name=f"py{b}_{hl}_{qb}")
                            nkt = (qb + 1) * (QB // KT)
                            for p0 in range(nkt // 2):
                                ps2 = s_psum.tile(
                                    [128, 1024], F32, tag="ps",
                                    name=f"ps{b}_{hl}_{qb}_{p0}",
                                )
                                for j in range(2):
                                    kt = 2 * p0 + j
                                    nc.tensor.matmul(
                                        ps2[:, j * 512 : (j + 1) * 512],
                                        kT[rb : rb + DH, cb + kt * KT : cb + (kt + 1) * KT],
                                        qT[rb : rb + DH, qs],
                                        start=True, stop=True,
                                    )
                                ex2 = epool.tile([128, 1024], BF16, tag="ex", name=f"ex{b}_{hl}_{qb}_{p0}")
                                nc.scalar.activation(ex2[:], ps2[:], AF.Exp)
                                for j in range(2):
                                    kt = 2 * p0 + j
                                    r = kt - qb * (QB // KT)
                                    if r >= 0:
                                        # diagonal-crossing tile: zero k > q
                                        nc.vector.tensor_mul(
                                            ex2[:, j * 512 : (j + 1) * 512],
                                            ex2[:, j * 512 : (j + 1) * 512],
                                            wmask[:, 512 - r * KT : 1024 - r * KT],
                                        )
                                for j in range(2):
                                    kt = 2 * p0 + j
                                    nc.tensor.matmul(
                                        py[:65],
                                        v_res[:, b % 2, kt, vlo:vhi],
                                        ex2[:, j * 512 : (j + 1) * 512],
                                        start=(kt == 0), stop=(kt == nkt - 1),
                                    )
                                # interleave one proj half-tile of prev batch
                                # or one V transpose of the next batch
                                if blocks >= 1 and prev is not None and proj_u < 32:
                                    emit_proj_unit(prev, proj_u)
                                    proj_u += 1
                                elif b + 1 < B and vt_kt < n_ktiles:
                                    emit_v_transpose(b + 1, vt_kt)
                                    vt_kt += 1
                            p = hl * 4 + qb
                            nc.vector.tensor_copy(
                                denw[:, p * QB : (p + 1) * QB], py[64:65, :]
                            )
                            nc.vector.tensor_copy(yT[rb : rb + DH, qs], py[:DH, :])
                            blocks += 1
                            if blocks == 1 and prev is not None:
                                emit_norm(prev, rec_prev)
                    while b + 1 < B and vt_kt < n_ktiles:
                        emit_v_transpose(b + 1, vt_kt)
                        vt_kt += 1
                    # scatter den rows to 8 partitions (compute engines can
                    # only write partition bases 0/32/64/96)
                    for p in range(8):
                        nc.sync.dma_start(
                            den[p : p + 1, :], denw[:, p * QB : (p + 1) * QB]
                        )
                    rec = dpool.tile([8, QB], F32, tag="rec", name=f"rec{b}")
                    rec_bf = dpool.tile([8, QB], BF16, tag="rec_bf", name=f"recb{b}")
                    with nc.allow_low_precision(reason="softmax denom recip"):
                        nc.vector.reciprocal_approx_fast(rec[:], den[:])
                        nc.vector.tensor_copy(rec_bf[:], rec[:])
                    return rec_bf

                # batch 0's V transposes up front; later batches' transposes
                # and norm/proj of b-1 ride inside attention(b)
                for kt in range(n_ktiles):
                    emit_v_transpose(0, kt)
                pending = None  # rec_bf of the batch awaiting norm+proj
                for b in range(B):
                    rec_bf = emit_attention(
                        b, b - 1 if pending is not None else None, pending
                    )
                    pending = rec_bf
                emit_norm(B - 1, pending)
                for u in range(32):
                    emit_proj_unit(B - 1, u, alt_pool=True)

    nc.compile()
    return nc


def _get_nc():
    global _CACHED_NC
    if _CACHED_NC is None:
        _CACHED_NC = _build()
    return _CACHED_NC


def kernel(x, W_qkv, b_qkv, W_proj, b_proj, _trace=False, _core_ids=None):
    global LAST_RESULT
    x = np.asarray(x, dtype=np.float32)
    W_qkv = np.asarray(W_qkv, dtype=np.float32)
    b_qkv = np.asarray(b_qkv, dtype=np.float32)
    W_proj = np.asarray(W_proj, dtype=np.float32)
    b_proj = np.asarray(b_proj, dtype=np.float32)

    xT = np.ascontiguousarray(x.reshape(NT, C).T).astype(NPBF)
    emat_np = np.zeros((8, 4, 128), dtype=NPBF)
    for qb in range(4):
        emat_np[qb, qb, :DH] = 1.0
        emat_np[4 + qb, qb, DH:] = 1.0
    core_ids = list(range(N_CORES)) if _core_ids is None else _core_ids
    in_maps = []
    for core in range(len(core_ids)):
        s = slice(core * DPC, (core + 1) * DPC)
        in_maps.append(
            {
                "xT": xT,
                "wq": np.ascontiguousarray(W_qkv[:, 0 * C + core * DPC : 0 * C + (core + 1) * DPC]).astype(NPBF),
                "wk": np.ascontiguousarray(W_qkv[:, 1 * C + core * DPC : 1 * C + (core + 1) * DPC]).astype(NPBF),
                "wv": np.ascontiguousarray(W_qkv[:, 2 * C + core * DPC : 2 * C + (core + 1) * DPC]).astype(NPBF),
                # device computes qT = psq*SCALE + bias, so pre-scale the q bias
                "bq": np.ascontiguousarray(b_qkv[0 * C + core * DPC : 0 * C + (core + 1) * DPC, None]) * np.float32(SCALE),
                "bk": np.ascontiguousarray(b_qkv[1 * C + core * DPC : 1 * C + (core + 1) * DPC, None]),
                "bv": np.ascontiguousarray(b_qkv[2 * C + core * DPC : 2 * C + (core + 1) * DPC, None]),
                "wp": np.ascontiguousarray(W_proj[s, :]).astype(NPBF),
                "emat": emat_np,
            }
        )

    nc = _get_nc()
    res = run_bass_kernel_spmd(nc, in_maps, core_ids, trace=_trace)
    LAST_RESULT = res

    acc = np.zeros((NT, C), dtype=np.float32)
    for r in res.results:
        acc += r["out"].astype(np.float32)
    acc += b_proj
    return acc.reshape(B, T, C).astype(np.float32)


# revision 24
# speedup vs baseline: 1.3187x; 1.1193x over previous
"""Causal self-attention layer (B=4, T=2048, C=1024, H=16) on 8 TRN2 NeuronCores.

Sharding: Megatron-style tensor parallel over heads — 2 heads per core.
Each core computes q/k/v projections for its 2 heads, causal attention with an
appended-ones column on V for softmax denominators, and a partial output
projection against its 128-row slice of W_proj. The host sums the 8 partial
projections and adds b_proj.

v2 changes vs the f32r baseline (652 us):
- All matmul operands bf16, converted on the HOST (x, weights, emat), so the
  device never casts x (removes ~68 us of DVE CAST work and halves input DMA).
- Scores for two k-tiles land in one 2-bank [128,1024] PSUM tile so each ACT
  exp covers both (ACT was the attention pacing engine at 687 ns/512-tile).
- DMA spread across the three DMA-capable issue engines (sync/scalar HWDGE +
  gpsimd SWDGE) instead of serializing everything on the sync queue.
- V transposed 128 rows at a time (both heads per instruction) into a resident
  [128, kt, 130] tile with two constant ones columns (written once).
- Denominator reciprocal via the single-instruction reciprocal_approx_fast
  (the exact DVE reciprocal measured 3.3 us per batch).
- Normalization + output projection of batch b deferred until after attention
  of batch b+1 is emitted, hiding the reciprocal chain and output DMA.
- Output partials in bf16 (halves output DMA; host sums partials in f32).
"""
import sys

sys.path.insert(0, "/opt/trn_rl_repo")

import ml_dtypes
import numpy as np

import concourse.bass as bass  # noqa: F401
from concourse import bacc
import concourse.mybir as mybir
import concourse.tile as tile
from concourse.bass_utils import run_bass_kernel_spmd
from concourse.masks import make_identity

B, T, C = 4, 2048, 1024
H, DH = 16, 64
N_CORES = 8
HPC = H // N_CORES          # heads per core = 2
DPC = HPC * DH              # head-dims per core = 128
NT = B * T                  # 8192 tokens
CH = C // 128               # 8 contraction chunks
QB = 512                    # q-block width (moving dim)
KT = 128                    # k-tile width (PE partition dim)
SCALE = 1.0 / 8.0           # 1/sqrt(DH)

F32 = mybir.dt.float32
BF16 = mybir.dt.bfloat16
AF = mybir.ActivationFunctionType
NPBF = ml_dtypes.bfloat16

_CACHED_NC = None
LAST_RESULT = None


def _build():
    nc = bacc.Bacc(None)

    xT = nc.dram_tensor("xT", [C, NT], BF16, kind="ExternalInput")
    wq = nc.dram_tensor("wq", [C, DPC], BF16, kind="ExternalInput")
    wk = nc.dram_tensor("wk", [C, DPC], BF16, kind="ExternalInput")
    wv = nc.dram_tensor("wv", [C, DPC], BF16, kind="ExternalInput")
    bq = nc.dram_tensor("bq", [DPC, 1], F32, kind="ExternalInput")
    bk = nc.dram_tensor("bk", [DPC, 1], F32, kind="ExternalInput")
    bv = nc.dram_tensor("bv", [DPC, 1], F32, kind="ExternalInput")
    wp = nc.dram_tensor("wp", [DPC, C], BF16, kind="ExternalInput")
    emat_in = nc.dram_tensor("emat", [8, 4, 128], BF16, kind="ExternalInput")
    out = nc.dram_tensor("out", [NT, C], BF16, kind="ExternalOutput")

    with tile.TileContext(nc) as tc:
        with (
            tc.tile_pool(name="const", bufs=1) as const,
            tc.tile_pool(name="res", bufs=1) as res,
        ):
            # --- constants ---
            ident = const.tile([128, 128], BF16, tag="ident")
            # sliding causal mask: wmask[k, u] = 1 iff k <= u - 512; a crossing
            # tile r multiplies by wmask[:, 512-128r : 1024-128r]
            wmask = const.tile([128, 1024], BF16, tag="wmask")
            emat = const.tile([8, 4, 128], BF16, tag="emat")
            with tc.tile_pool(name="cstage", bufs=1) as cstage:
                ident_s = cstage.tile([128, 128], F32, tag="ident_s")
                make_identity(nc, ident_s[:])
                nc.vector.tensor_copy(ident[:], ident_s[:])

                wmask_s = cstage.tile([128, 1024], F32, tag="wmask_s")
                nc.gpsimd.memset(wmask_s[:], 0.0)
                nc.gpsimd.affine_select(
                    out=wmask_s[:],
                    in_=wmask_s[:],
                    compare_op=mybir.AluOpType.is_gt,
                    fill=1.0,
                    base=512,
                    # keep 0 where (512 + k - u) > 0, fill 1 where k <= u - 512
                    pattern=[[-1, 1024]],
                    channel_multiplier=1,
                )
                nc.vector.tensor_copy(wmask[:], wmask_s[:])

                nc.sync.dma_start(emat[:], emat_in[:])

            bq_t = const.tile([DPC, 1], F32, tag="bq")
            bk_t = const.tile([DPC, 1], F32, tag="bk")
            bv_t = const.tile([DPC, 1], F32, tag="bv")
            nc.sync.dma_start(bq_t[:], bq[:])
            nc.sync.dma_start(bk_t[:], bk[:])
            nc.sync.dma_start(bv_t[:], bv[:])

            # weights straight into SBUF as bf16 (host pre-converted)
            wq_r = const.tile([128, CH, DPC], BF16, tag="wq_r")
            wk_r = const.tile([128, CH, DPC], BF16, tag="wk_r")
            wv_r = const.tile([128, CH, DPC], BF16, tag="wv_r")
            wp_r = const.tile([DPC, C], BF16, tag="wp_r")
            for w_in, w_dst in ((wq, wq_r), (wk, wk_r), (wv, wv_r)):
                w_re = w_in.rearrange("(c p) n -> p c n", p=128)
                nc.scalar.dma_start(w_dst[:], w_re)
            nc.sync.dma_start(wp_r[:], wp[:])

            # --- residents ---
            qT = res.tile([DPC, NT], BF16, tag="qT")
            kT = res.tile([DPC, NT], BF16, tag="kT")
            vT = res.tile([DPC, NT], BF16, tag="vT")
            yT = res.tile([DPC, NT], BF16, tag="yT")
            # per-batch transposed V (double-buffered so batch b's DMA
            # transposes overlap batch b-1's attention):
            # [tok128, buf, kt, 0:64]=h0 dims, 64=ones, [65:129]=h1, 129=ones
            v_res = res.tile([128, 2, T // KT, 130], BF16, tag="v_res")
            nc.gpsimd.memset(v_res[:, :, :, 64:65], 1.0)
            nc.gpsimd.memset(v_res[:, :, :, 129:130], 1.0)

            # ====== fused pipeline: qkv(b+1) + attention(b) + proj(b-1) ======
            xT_re = xT.rearrange("(c p) t -> c p t", p=128)
            dma_engines = [nc.sync, nc.scalar, nc.gpsimd]
            with (
                tc.tile_pool(name="xpool", bufs=24) as xpool,
                tc.tile_pool(name="epool", bufs=6) as epool,
                tc.tile_pool(name="dpool", bufs=2) as dpool,
                tc.tile_pool(name="opool", bufs=6) as opool,
                tc.tile_pool(name="s_psum", bufs=2, space="PSUM") as s_psum,
                tc.tile_pool(name="y_psum", bufs=2, space="PSUM") as y_psum,
                tc.tile_pool(name="m_psum", bufs=1, space="PSUM") as m_psum,
                tc.tile_pool(name="p_psum", bufs=1, space="PSUM") as p_psum,
            ):
                n_ktiles = T // KT  # 16

                def emit_qkv_tile(tt):
                    """QKV projection for one 512-token tile: q|k share one
                    wide 2-bank score-pool tile, v rides the py pool."""
                    ts_ = slice(tt * QB, (tt + 1) * QB)
                    xrs = []
                    for c in range(CH):
                        xs = xpool.tile([128, QB], BF16, tag="xs")
                        eng = dma_engines[(tt * CH + c) % 3]
                        eng.dma_start(xs[:], xT_re[c, :, ts_])
                        xrs.append(xs)
                    psqk = s_psum.tile([128, 1024], F32, tag="ps", name=f"qk{tt}")
                    psv = y_psum.tile([128, QB], F32, tag="py", name=f"v{tt}")
                    for half, w_r in ((0, wq_r), (1, wk_r)):
                        for c in range(CH):
                            nc.tensor.matmul(
                                psqk[:, half * 512 : (half + 1) * 512],
                                w_r[:, c, :], xrs[c][:],
                                start=(c == 0), stop=(c == CH - 1),
                            )
                    for c in range(CH):
                        nc.tensor.matmul(
                            psv[:], wv_r[:, c, :], xrs[c][:],
                            start=(c == 0), stop=(c == CH - 1),
                        )
                    # copy out of PSUM (+bias; 1/sqrt(dh) is folded into the
                    # host-side Wq, exact since it is a power of two)
                    nc.vector.tensor_scalar_add(qT[:, ts_], psqk[:, :512], bq_t[:])
                    nc.scalar.activation(kT[:, ts_], psqk[:, 512:], AF.Identity, bias=bk_t[:])
                    nc.vector.tensor_scalar_add(vT[:, ts_], psv[:], bv_t[:])

                def emit_v_transpose(b, kt):
                    """PE-transpose one 128-token V tile (both heads at once)
                    for batch b into v_res[:, b%2]."""
                    cb = b * T
                    pt = m_psum.tile([128, 128], BF16, tag="mps", name=f"pt{b}_{kt}")
                    nc.tensor.transpose(
                        pt[:], vT[:, cb + kt * KT : cb + (kt + 1) * KT], ident[:]
                    )
                    nc.vector.tensor_copy(v_res[:, b % 2, kt, 0:64], pt[:, 0:64])
                    nc.vector.tensor_copy(v_res[:, b % 2, kt, 65:129], pt[:, 64:128])

                def emit_norm(b, rec_bf):
                    cb = b * T
                    for qb in range(T // QB):
                        qs = slice(cb + qb * QB, cb + (qb + 1) * QB)
                        pb = s_psum.tile([128, 1024], F32, tag="ps", name=f"pb{b}_{qb}")
                        nc.tensor.matmul(
                            pb[:, :512], emat[:, qb, :], rec_bf[:],
                            start=True, stop=True,
                        )
                        nc.vector.tensor_mul(yT[:, qs], yT[:, qs], pb[:, :512])

                def emit_proj_unit(b, u, alt_pool=False):
                    """One [128 tok, 512 col] half-tile of batch b's partial
                    projection: single matmul + evac + DMA. With alt_pool
                    (the tail, when attention PSUM is idle) rotate across the
                    other pools for depth instead of the single pp slot."""
                    tt, half = u // 2, u % 2
                    trow = b * T + tt * 128
                    if not alt_pool or u % 4 == 0:
                        pp = p_psum.tile([128, 512], F32, tag="pp", name=f"pp{b}_{u}")[:]
                    elif u % 2 == 1:
                        pp = s_psum.tile([128, 1024], F32, tag="ps", name=f"pp{b}_{u}")[:, :512]
                    else:
                        pp = y_psum.tile([128, 512], F32, tag="py", name=f"pp{b}_{u}")[:]
                    nc.tensor.matmul(
                        pp,
                        yT[:, trow : trow + 128],
                        wp_r[:, half * 512 : (half + 1) * 512],
                        start=True, stop=True,
                    )
                    os_ = opool.tile([128, 512], BF16, tag="os", name=f"os{b}_{u}")
                    # alternate evac engine: ACT is exp-loaded during the
                    # overlapped attention, DVE has more slack
                    if u % 2 == 0:
                        nc.vector.tensor_copy(os_[:], pp)
                    else:
                        nc.scalar.copy(os_[:], pp)
                    eng = nc.sync if u % 2 == 0 else nc.gpsimd
                    eng.dma_start(
                        out[trow : trow + 128, half * 512 : (half + 1) * 512],
                        os_[:],
                    )

                def emit_attention(b, prev, rec_prev):
                    """Scores+exp+PV for batch b with everything else riding
                    in its pair slots: batch b+1's QKV tiles (pairs 1/11/21/
                    31), prev's norm (after block 0) and 32 projection
                    half-tiles, and batch b+1's V transposes (late pairs).
                    Attention is ACT-exp paced, so the PE slack per pair
                    absorbs the extra matmuls."""
                    cb = b * T
                    denw = dpool.tile([1, 8 * QB], F32, tag="denw", name=f"denw{b}")
                    den = dpool.tile([8, QB], F32, tag="den", name=f"den{b}")
                    qkv_slots = {1: 0, 11: 1, 21: 2, 31: 3}
                    proj_u = 0
                    vt_kt = 0
                    pair = 0
                    blocks = 0
                    for hl in range(HPC):
                        rb = hl * DH
                        vlo, vhi = (0, 65) if hl == 0 else (65, 130)
                        for qb in range(T // QB):
                            qs = slice(cb + qb * QB, cb + (qb + 1) * QB)
                            py = y_psum.tile([128, QB], F32, tag="py", name=f"py{b}_{hl}_{qb}")
                            nkt = (qb + 1) * (QB // KT)
                            for p0 in range(nkt // 2):
                                ps2 = s_psum.tile(
                                    [128, 1024], F32, tag="ps",
                                    name=f"ps{b}_{hl}_{qb}_{p0}",
                                )
                                for j in range(2):
                                    kt = 2 * p0 + j
                                    nc.tensor.matmul(
                                        ps2[:, j * 512 : (j + 1) * 512],
                                        kT[rb : rb + DH, cb + kt * KT : cb + (kt + 1) * KT],
                                        qT[rb : rb + DH, qs],
                                        start=True, stop=True,
                                    )
                                ex2 = epool.tile([128, 1024], BF16, tag="ex", name=f"ex{b}_{hl}_{qb}_{p0}")
                                nc.scalar.activation(ex2[:], ps2[:], AF.Exp)
                                for j in range(2):
                                    kt = 2 * p0 + j
                                    r = kt - qb * (QB // KT)
                                    if r >= 0:
                                        # diagonal-crossing tile: zero k > q
                                        nc.vector.tensor_mul(
                                            ex2[:, j * 512 : (j + 1) * 512],
                                            ex2[:, j * 512 : (j + 1) * 512],
                                            wmask[:, 512 - r * KT : 1024 - r * KT],
                                        )
                                for j in range(2):
                                    kt = 2 * p0 + j
                                    nc.tensor.matmul(
                                        py[:65],
                                        v_res[:, b % 2, kt, vlo:vhi],
                                        ex2[:, j * 512 : (j + 1) * 512],
                                        start=(kt == 0), stop=(kt == nkt - 1),
                                    )
                                # one rider unit per pair
                                if b + 1 < B and pair in qkv_slots:
                                    emit_qkv_tile(4 * (b + 1) + qkv_slots[pair])
                                elif blocks >= 1 and prev is not None and proj_u < 32:
                                    emit_proj_unit(prev, proj_u)
                                    proj_u += 1
                                elif b + 1 < B and vt_kt < n_ktiles and pair >= 32:
                                    emit_v_transpose(b + 1, vt_kt)
                                    vt_kt += 1
                                pair += 1
                            p = hl * 4 + qb
                            nc.vector.tensor_copy(
                                denw[:, p * QB : (p + 1) * QB], py[64:65, :]
                            )
                            nc.vector.tensor_copy(yT[rb : rb + DH, qs], py[:DH, :])
                            blocks += 1
                            if blocks == 1 and prev is not None:
                                emit_norm(prev, rec_prev)
                    while b + 1 < B and vt_kt < n_ktiles:
                        emit_v_transpose(b + 1, vt_kt)
                        vt_kt += 1
                    # scatter den rows to 8 partitions (compute engines can
                    # only write partition bases 0/32/64/96)
                    for p in range(8):
                        nc.sync.dma_start(
                            den[p : p + 1, :], denw[:, p * QB : (p + 1) * QB]
                        )
                    rec = dpool.tile([8, QB], F32, tag="rec", name=f"rec{b}")
                    rec_bf = dpool.tile([8, QB], BF16, tag="rec_bf", name=f"recb{b}")
                    with nc.allow_low_precision(reason="softmax denom recip"):
                        nc.vector.reciprocal_approx_fast(rec[:], den[:])
                        nc.vector.tensor_copy(rec_bf[:], rec[:])
                    return rec_bf

                # head: batch 0's QKV and V transposes stand alone; the rest
                # of the pipeline rides inside attention
                for tt in range(4):
                    emit_qkv_tile(tt)
                for kt in range(n_ktiles):
                    emit_v_transpose(0, kt)
                pending = None  # rec_bf of the batch awaiting norm+proj
                for b in range(B):
                    rec_bf = emit_attention(
                        b, b - 1 if pending is not None else None, pending
                    )
                    pending = rec_bf
                emit_norm(B - 1, pending)
                for u in range(32):
                    emit_proj_unit(B - 1, u, alt_pool=True)

    nc.compile()
    return nc


def _get_nc():
    global _CACHED_NC
    if _CACHED_NC is None:
        _CACHED_NC = _build()
    return _CACHED_NC


def kernel(x, W_qkv, b_qkv, W_proj, b_proj, _trace=False, _core_ids=None):
    global LAST_RESULT
    x = np.asarray(x, dtype=np.float32)
    W_qkv = np.asarray(W_qkv, dtype=np.float32)
    b_qkv = np.asarray(b_qkv, dtype=np.float32)
    W_proj = np.asarray(W_proj, dtype=np.float32)
    b_proj = np.asarray(b_proj, dtype=np.float32)

    xT = np.ascontiguousarray(x.reshape(NT, C).T).astype(NPBF)
    emat_np = np.zeros((8, 4, 128), dtype=NPBF)
    for qb in range(4):
        emat_np[qb, qb, :DH] = 1.0
        emat_np[4 + qb, qb, DH:] = 1.0
    core_ids = list(range(N_CORES)) if _core_ids is None else _core_ids
    in_maps = []
    for core in range(len(core_ids)):
        s = slice(core * DPC, (core + 1) * DPC)
        in_maps.append(
            {
                "xT": xT,
                # 1/sqrt(dh) folded into Wq and bq (exact: power of two)
                "wq": (np.ascontiguousarray(W_qkv[:, 0 * C + core * DPC : 0 * C + (core + 1) * DPC]) * np.float32(SCALE)).astype(NPBF),
                "wk": np.ascontiguousarray(W_qkv[:, 1 * C + core * DPC : 1 * C + (core + 1) * DPC]).astype(NPBF),
                "wv": np.ascontiguousarray(W_qkv[:, 2 * C + core * DPC : 2 * C + (core + 1) * DPC]).astype(NPBF),
                "bq": np.ascontiguousarray(b_qkv[0 * C + core * DPC : 0 * C + (core + 1) * DPC, None]) * np.float32(SCALE),
                "bk": np.ascontiguousarray(b_qkv[1 * C + core * DPC : 1 * C + (core + 1) * DPC, None]),
                "bv": np.ascontiguousarray(b_qkv[2 * C + core * DPC : 2 * C + (core + 1) * DPC, None]),
                "wp": np.ascontiguousarray(W_proj[s, :]).astype(NPBF),
                "emat": emat_np,
            }
        )

    nc = _get_nc()
    res = run_bass_kernel_spmd(nc, in_maps, core_ids, trace=_trace)
    LAST_RESULT = res

    acc = np.zeros((NT, C), dtype=np.float32)
    for r in res.results:
        acc += r["out"].astype(np.float32)
    acc += b_proj
    return acc.reshape(B, T, C).astype(np.float32)


# revision 28
# speedup vs baseline: 1.3537x; 1.0266x over previous
"""Causal self-attention layer (B=4, T=2048, C=1024, H=16) on 8 TRN2 NeuronCores.

Sharding: Megatron-style tensor parallel over heads — 2 heads per core.
Each core computes q/k/v projections for its 2 heads, causal attention with an
appended-ones column on V for softmax denominators, and a partial output
projection against its 128-row slice of W_proj. The host sums the 8 partial
projections and adds b_proj.

v2 changes vs the f32r baseline (652 us):
- All matmul operands bf16, converted on the HOST (x, weights, emat), so the
  device never casts x (removes ~68 us of DVE CAST work and halves input DMA).
- Scores for two k-tiles land in one 2-bank [128,1024] PSUM tile so each ACT
  exp covers both (ACT was the attention pacing engine at 687 ns/512-tile).
- DMA spread across the three DMA-capable issue engines (sync/scalar HWDGE +
  gpsimd SWDGE) instead of serializing everything on the sync queue.
- V transposed 128 rows at a time (both heads per instruction) into a resident
  [128, kt, 130] tile with two constant ones columns (written once).
- Denominator reciprocal via the single-instruction reciprocal_approx_fast
  (the exact DVE reciprocal measured 3.3 us per batch).
- Normalization + output projection of batch b deferred until after attention
  of batch b+1 is emitted, hiding the reciprocal chain and output DMA.
- Output partials in bf16 (halves output DMA; host sums partials in f32).
"""
import sys

sys.path.insert(0, "/opt/trn_rl_repo")

import ml_dtypes
import numpy as np

import concourse.bass as bass  # noqa: F401
from concourse import bacc
import concourse.mybir as mybir
import concourse.tile as tile
from concourse.bass_utils import run_bass_kernel_spmd
from concourse.masks import make_identity

B, T, C = 4, 2048, 1024
H, DH = 16, 64
N_CORES = 8
HPC = H // N_CORES          # heads per core = 2
DPC = HPC * DH              # head-dims per core = 128
NT = B * T                  # 8192 tokens
CH = C // 128               # 8 contraction chunks
QB = 512                    # q-block width (moving dim)
KT = 128                    # k-tile width (PE partition dim)
SCALE = 1.0 / 8.0           # 1/sqrt(DH)

F32 = mybir.dt.float32
BF16 = mybir.dt.bfloat16
AF = mybir.ActivationFunctionType
NPBF = ml_dtypes.bfloat16

_CACHED_NC = None
LAST_RESULT = None


def _build():
    nc = bacc.Bacc(None)

    xT = nc.dram_tensor("xT", [C, NT], BF16, kind="ExternalInput")
    wq = nc.dram_tensor("wq", [C, DPC], BF16, kind="ExternalInput")
    wk = nc.dram_tensor("wk", [C, DPC], BF16, kind="ExternalInput")
    wv = nc.dram_tensor("wv", [C, DPC], BF16, kind="ExternalInput")
    bq = nc.dram_tensor("bq", [DPC, 1], F32, kind="ExternalInput")
    bk = nc.dram_tensor("bk", [DPC, 1], F32, kind="ExternalInput")
    bv = nc.dram_tensor("bv", [DPC, 1], F32, kind="ExternalInput")
    wp = nc.dram_tensor("wp", [DPC, C], BF16, kind="ExternalInput")
    emat_in = nc.dram_tensor("emat", [8, 4, 128], BF16, kind="ExternalInput")
    out = nc.dram_tensor("out", [NT, C], BF16, kind="ExternalOutput")

    with tile.TileContext(nc) as tc:
        with (
            tc.tile_pool(name="const", bufs=1) as const,
            tc.tile_pool(name="res", bufs=1) as res,
        ):
            # --- constants ---
            ident = const.tile([128, 128], BF16, tag="ident")
            # sliding causal mask: wmask[k, u] = 1 iff k <= u - 512; a crossing
            # tile r multiplies by wmask[:, 512-128r : 1024-128r]
            wmask = const.tile([128, 1024], BF16, tag="wmask")
            emat = const.tile([8, 4, 128], BF16, tag="emat")
            with tc.tile_pool(name="cstage", bufs=1) as cstage:
                ident_s = cstage.tile([128, 128], F32, tag="ident_s")
                make_identity(nc, ident_s[:])
                nc.vector.tensor_copy(ident[:], ident_s[:])

                wmask_s = cstage.tile([128, 1024], F32, tag="wmask_s")
                nc.gpsimd.memset(wmask_s[:], 0.0)
                nc.gpsimd.affine_select(
                    out=wmask_s[:],
                    in_=wmask_s[:],
                    compare_op=mybir.AluOpType.is_gt,
                    fill=1.0,
                    base=512,
                    # keep 0 where (512 + k - u) > 0, fill 1 where k <= u - 512
                    pattern=[[-1, 1024]],
                    channel_multiplier=1,
                )
                nc.vector.tensor_copy(wmask[:], wmask_s[:])

                nc.sync.dma_start(emat[:], emat_in[:])

            bq_t = const.tile([DPC, 1], F32, tag="bq")
            bk_t = const.tile([DPC, 1], F32, tag="bk")
            bv_t = const.tile([DPC, 1], F32, tag="bv")
            nc.sync.dma_start(bq_t[:], bq[:])
            nc.sync.dma_start(bk_t[:], bk[:])
            nc.sync.dma_start(bv_t[:], bv[:])

            # weights straight into SBUF as bf16 (host pre-converted)
            wq_r = const.tile([128, CH, DPC], BF16, tag="wq_r")
            wk_r = const.tile([128, CH, DPC], BF16, tag="wk_r")
            wv_r = const.tile([128, CH, DPC], BF16, tag="wv_r")
            wp_r = const.tile([DPC, C], BF16, tag="wp_r")
            for w_in, w_dst in ((wq, wq_r), (wk, wk_r), (wv, wv_r)):
                w_re = w_in.rearrange("(c p) n -> p c n", p=128)
                nc.scalar.dma_start(w_dst[:], w_re)
            nc.sync.dma_start(wp_r[:], wp[:])

            # --- residents ---
            qT = res.tile([DPC, NT], BF16, tag="qT")
            kT = res.tile([DPC, NT], BF16, tag="kT")
            vT = res.tile([DPC, NT], BF16, tag="vT")
            yT = res.tile([DPC, NT], BF16, tag="yT")
            # per-batch transposed V (double-buffered so batch b's DMA
            # transposes overlap batch b-1's attention):
            # [tok128, buf, kt, 0:64]=h0 dims, 64=ones, [65:129]=h1, 129=ones
            v_res = res.tile([128, 2, T // KT, 130], BF16, tag="v_res")
            nc.gpsimd.memset(v_res[:, :, :, 64:65], 1.0)
            nc.gpsimd.memset(v_res[:, :, :, 129:130], 1.0)

            # ====== fused pipeline: qkv(b+1) + attention(b) + proj(b-1) ======
            xT_re = xT.rearrange("(c p) t -> c p t", p=128)
            dma_engines = [nc.sync, nc.scalar, nc.gpsimd]
            with (
                tc.tile_pool(name="xpool", bufs=24) as xpool,
                tc.tile_pool(name="epool", bufs=6) as epool,
                tc.tile_pool(name="dpool", bufs=2) as dpool,
                tc.tile_pool(name="opool", bufs=6) as opool,
                tc.tile_pool(name="s_psum", bufs=2, space="PSUM") as s_psum,
                tc.tile_pool(name="y_psum", bufs=2, space="PSUM") as y_psum,
                tc.tile_pool(name="m_psum", bufs=1, space="PSUM") as m_psum,
                tc.tile_pool(name="p_psum", bufs=1, space="PSUM") as p_psum,
            ):
                n_ktiles = T // KT  # 16

                def emit_qkv_tile(tt, head=False):
                    """QKV projection for one 512-token tile: q|k share one
                    wide 2-bank score-pool tile, v rides the py pool. Rider
                    tiles (mid-attention) avoid the scalar DMA queue so the
                    ACT sequencer stays on exp dispatch."""
                    ts_ = slice(tt * QB, (tt + 1) * QB)
                    engines = dma_engines if head else [nc.sync, nc.gpsimd]
                    xrs = []
                    for c in range(CH):
                        xs = xpool.tile([128, QB], BF16, tag="xs")
                        eng = engines[(tt * CH + c) % len(engines)]
                        eng.dma_start(xs[:], xT_re[c, :, ts_])
                        xrs.append(xs)
                    psqk = s_psum.tile([128, 1024], F32, tag="ps", name=f"qk{tt}")
                    psv = y_psum.tile([128, QB], F32, tag="py", name=f"v{tt}")
                    for half, w_r in ((0, wq_r), (1, wk_r)):
                        for c in range(CH):
                            nc.tensor.matmul(
                                psqk[:, half * 512 : (half + 1) * 512],
                                w_r[:, c, :], xrs[c][:],
                                start=(c == 0), stop=(c == CH - 1),
                            )
                    for c in range(CH):
                        nc.tensor.matmul(
                            psv[:], wv_r[:, c, :], xrs[c][:],
                            start=(c == 0), stop=(c == CH - 1),
                        )
                    # copy out of PSUM (+bias; 1/sqrt(dh) is folded into the
                    # host-side Wq, exact since it is a power of two)
                    nc.vector.tensor_scalar_add(qT[:, ts_], psqk[:, :512], bq_t[:])
                    nc.scalar.activation(kT[:, ts_], psqk[:, 512:], AF.Identity, bias=bk_t[:])
                    nc.vector.tensor_scalar_add(vT[:, ts_], psv[:], bv_t[:])

                def emit_v_transpose(b, kt):
                    """PE-transpose one 128-token V tile (both heads at once)
                    for batch b into v_res[:, b%2]."""
                    cb = b * T
                    pt = m_psum.tile([128, 128], BF16, tag="mps", name=f"pt{b}_{kt}")
                    nc.tensor.transpose(
                        pt[:], vT[:, cb + kt * KT : cb + (kt + 1) * KT], ident[:]
                    )
                    nc.vector.tensor_copy(v_res[:, b % 2, kt, 0:64], pt[:, 0:64])
                    nc.vector.tensor_copy(v_res[:, b % 2, kt, 65:129], pt[:, 64:128])

                def emit_norm(b, rec_bf):
                    cb = b * T
                    for qb in range(T // QB):
                        qs = slice(cb + qb * QB, cb + (qb + 1) * QB)
                        pb = s_psum.tile([128, 1024], F32, tag="ps", name=f"pb{b}_{qb}")
                        nc.tensor.matmul(
                            pb[:, :512], emat[:, qb, :], rec_bf[:],
                            start=True, stop=True,
                        )
                        nc.vector.tensor_mul(yT[:, qs], yT[:, qs], pb[:, :512])

                def emit_proj_unit(b, u, alt_pool=False):
                    """One [128 tok, 512 col] half-tile of batch b's partial
                    projection: single matmul + evac + DMA. With alt_pool
                    (the tail, when attention PSUM is idle) rotate across the
                    other pools for depth instead of the single pp slot."""
                    tt, half = u // 2, u % 2
                    trow = b * T + tt * 128
                    if not alt_pool or u % 4 == 0:
                        pp = p_psum.tile([128, 512], F32, tag="pp", name=f"pp{b}_{u}")[:]
                    elif u % 2 == 1:
                        pp = s_psum.tile([128, 1024], F32, tag="ps", name=f"pp{b}_{u}")[:, :512]
                    else:
                        pp = y_psum.tile([128, 512], F32, tag="py", name=f"pp{b}_{u}")[:]
                    nc.tensor.matmul(
                        pp,
                        yT[:, trow : trow + 128],
                        wp_r[:, half * 512 : (half + 1) * 512],
                        start=True, stop=True,
                    )
                    os_ = opool.tile([128, 512], BF16, tag="os", name=f"os{b}_{u}")
                    # evac mostly on DVE: ACT is exp-paced during the
                    # overlapped attention
                    if u % 4 == 1:
                        nc.scalar.copy(os_[:], pp)
                    else:
                        nc.vector.tensor_copy(os_[:], pp)
                    eng = nc.sync if u % 2 == 0 else nc.gpsimd
                    eng.dma_start(
                        out[trow : trow + 128, half * 512 : (half + 1) * 512],
                        os_[:],
                    )

                def emit_attention(b, prev, rec_prev):
                    """Scores+exp+PV for batch b with everything else riding
                    in its pair slots: batch b+1's QKV tiles (pairs 1/11/21/
                    31), prev's norm (after block 0) and 32 projection
                    half-tiles, and batch b+1's V transposes (late pairs).
                    Attention is ACT-exp paced, so the PE slack per pair
                    absorbs the extra matmuls."""
                    cb = b * T
                    denw = dpool.tile([1, 8 * QB], F32, tag="denw", name=f"denw{b}")
                    den = dpool.tile([8, QB], F32, tag="den", name=f"den{b}")
                    # 20 quads per batch, two rider slots each; QKV tiles of
                    # b+1 go in post-PV slots spread across the batch
                    qkv_slots = {1: 0, 6: 1, 11: 2, 16: 3}
                    proj_u = 0
                    vt_kt = 0
                    quad = 0
                    blocks = 0

                    def rider(post_pv):
                        nonlocal proj_u, vt_kt
                        if post_pv and b + 1 < B and quad in qkv_slots:
                            emit_qkv_tile(4 * (b + 1) + qkv_slots[quad])
                        elif blocks >= 1 and prev is not None and proj_u < 32:
                            emit_proj_unit(prev, proj_u)
                            proj_u += 1
                        elif b + 1 < B and vt_kt < n_ktiles and quad >= 16:
                            emit_v_transpose(b + 1, vt_kt)
                            vt_kt += 1

                    for hl in range(HPC):
                        rb = hl * DH
                        vlo, vhi = (0, 65) if hl == 0 else (65, 130)
                        for qb in range(T // QB):
                            qs = slice(cb + qb * QB, cb + (qb + 1) * QB)
                            py = y_psum.tile([128, QB], F32, tag="py", name=f"py{b}_{hl}_{qb}")
                            nkt = (qb + 1) * (QB // KT)
                            for q0 in range(nkt // 4):
                                # 4 score singles into two 2-bank tiles, then
                                # 2 wide exps, masks, and a 4-long PV run
                                pss, exs = [], []
                                for p0 in (2 * q0, 2 * q0 + 1):
                                    ps2 = s_psum.tile(
                                        [128, 1024], F32, tag="ps",
                                        name=f"ps{b}_{hl}_{qb}_{p0}",
                                    )
                                    for j in range(2):
                                        kt = 2 * p0 + j
                                        nc.tensor.matmul(
                                            ps2[:, j * 512 : (j + 1) * 512],
                                            kT[rb : rb + DH, cb + kt * KT : cb + (kt + 1) * KT],
                                            qT[rb : rb + DH, qs],
                                            start=True, stop=True,
                                        )
                                    pss.append(ps2)
                                for i, p0 in enumerate((2 * q0, 2 * q0 + 1)):
                                    ex2 = epool.tile([128, 1024], BF16, tag="ex", name=f"ex{b}_{hl}_{qb}_{p0}")
                                    nc.scalar.activation(ex2[:], pss[i][:], AF.Exp)
                                    for j in range(2):
                                        kt = 2 * p0 + j
                                        r = kt - qb * (QB // KT)
                                        if r >= 0:
                                            # diagonal-crossing: zero k > q
                                            nc.vector.tensor_mul(
                                                ex2[:, j * 512 : (j + 1) * 512],
                                                ex2[:, j * 512 : (j + 1) * 512],
                                                wmask[:, 512 - r * KT : 1024 - r * KT],
                                            )
                                    exs.append(ex2)
                                rider(post_pv=False)
                                for i in range(4):
                                    kt = 4 * q0 + i
                                    nc.tensor.matmul(
                                        py[:65],
                                        v_res[:, b % 2, kt, vlo:vhi],
                                        exs[i // 2][:, (i % 2) * 512 : (i % 2 + 1) * 512],
                                        start=(kt == 0), stop=(kt == nkt - 1),
                                    )
                                rider(post_pv=True)
                                quad += 1
                            p = hl * 4 + qb
                            nc.vector.tensor_copy(
                                denw[:, p * QB : (p + 1) * QB], py[64:65, :]
                            )
                            nc.vector.tensor_copy(yT[rb : rb + DH, qs], py[:DH, :])
                            blocks += 1
                            if blocks == 1 and prev is not None:
                                emit_norm(prev, rec_prev)
                    while b + 1 < B and vt_kt < n_ktiles:
                        emit_v_transpose(b + 1, vt_kt)
                        vt_kt += 1
                    # scatter den rows to 8 partitions (compute engines can
                    # only write partition bases 0/32/64/96)
                    for p in range(8):
                        nc.sync.dma_start(
                            den[p : p + 1, :], denw[:, p * QB : (p + 1) * QB]
                        )
                    rec = dpool.tile([8, QB], F32, tag="rec", name=f"rec{b}")
                    rec_bf = dpool.tile([8, QB], BF16, tag="rec_bf", name=f"recb{b}")
                    with nc.allow_low_precision(reason="softmax denom recip"):
                        nc.vector.reciprocal_approx_fast(rec[:], den[:])
                        nc.vector.tensor_copy(rec_bf[:], rec[:])
                    return rec_bf

                # head: batch 0's QKV and V transposes stand alone; the rest
                # of the pipeline rides inside attention
                for tt in range(4):
                    emit_qkv_tile(tt, head=True)
                    for kt in range(4 * tt, 4 * tt + 4):
                        emit_v_transpose(0, kt)
                pending = None  # rec_bf of the batch awaiting norm+proj
                for b in range(B):
                    rec_bf = emit_attention(
                        b, b - 1 if pending is not None else None, pending
                    )
                    pending = rec_bf
                emit_norm(B - 1, pending)
                for u in range(32):
                    emit_proj_unit(B - 1, u, alt_pool=True)

    nc.compile()
    return nc


def _get_nc():
    global _CACHED_NC
    if _CACHED_NC is None:
        _CACHED_NC = _build()
    return _CACHED_NC


def kernel(x, W_qkv, b_qkv, W_proj, b_proj, _trace=False, _core_ids=None):
    global LAST_RESULT
    x = np.asarray(x, dtype=np.float32)
    W_qkv = np.asarray(W_qkv, dtype=np.float32)
    b_qkv = np.asarray(b_qkv, dtype=np.float32)
    W_proj = np.asarray(W_proj, dtype=np.float32)
    b_proj = np.asarray(b_proj, dtype=np.float32)

    xT = np.ascontiguousarray(x.reshape(NT, C).T).astype(NPBF)
    emat_np = np.zeros((8, 4, 128), dtype=NPBF)
    for qb in range(4):
        emat_np[qb, qb, :DH] = 1.0
        emat_np[4 + qb, qb, DH:] = 1.0
    core_ids = list(range(N_CORES)) if _core_ids is None else _core_ids
    in_maps = []
    for core in range(len(core_ids)):
        s = slice(core * DPC, (core + 1) * DPC)
        in_maps.append(
            {
                "xT": xT,
                # 1/sqrt(dh) folded into Wq and bq (exact: power of two)
                "wq": (np.ascontiguousarray(W_qkv[:, 0 * C + core * DPC : 0 * C + (core + 1) * DPC]) * np.float32(SCALE)).astype(NPBF),
                "wk": np.ascontiguousarray(W_qkv[:, 1 * C + core * DPC : 1 * C + (core + 1) * DPC]).astype(NPBF),
                "wv": np.ascontiguousarray(W_qkv[:, 2 * C + core * DPC : 2 * C + (core + 1) * DPC]).astype(NPBF),
                "bq": np.ascontiguousarray(b_qkv[0 * C + core * DPC : 0 * C + (core + 1) * DPC, None]) * np.float32(SCALE),
                "bk": np.ascontiguousarray(b_qkv[1 * C + core * DPC : 1 * C + (core + 1) * DPC, None]),
                "bv": np.ascontiguousarray(b_qkv[2 * C + core * DPC : 2 * C + (core + 1) * DPC, None]),
                "wp": np.ascontiguousarray(W_proj[s, :]).astype(NPBF),
                "emat": emat_np,
            }
        )

    nc = _get_nc()
    res = run_bass_kernel_spmd(nc, in_maps, core_ids, trace=_trace)
    LAST_RESULT = res

    acc = np.zeros((NT, C), dtype=np.float32)
    for r in res.results:
        acc += r["out"].astype(np.float32)
    acc += b_proj
    return acc.reshape(B, T, C).astype(np.float32)


# revision 30
# speedup vs baseline: 1.4165x; 1.0464x over previous
"""Causal self-attention layer (B=4, T=2048, C=1024, H=16) on 8 TRN2 NeuronCores.

Sharding: Megatron-style tensor parallel over heads — 2 heads per core.
Each core computes q/k/v projections for its 2 heads, causal attention with an
appended-ones column on V for softmax denominators, and a partial output
projection against its 128-row slice of W_proj. The host sums the 8 partial
projections and adds b_proj.

v2 changes vs the f32r baseline (652 us):
- All matmul operands bf16, converted on the HOST (x, weights, emat), so the
  device never casts x (removes ~68 us of DVE CAST work and halves input DMA).
- Scores for two k-tiles land in one 2-bank [128,1024] PSUM tile so each ACT
  exp covers both (ACT was the attention pacing engine at 687 ns/512-tile).
- DMA spread across the three DMA-capable issue engines (sync/scalar HWDGE +
  gpsimd SWDGE) instead of serializing everything on the sync queue.
- V transposed 128 rows at a time (both heads per instruction) into a resident
  [128, kt, 130] tile with two constant ones columns (written once).
- Denominator reciprocal via the single-instruction reciprocal_approx_fast
  (the exact DVE reciprocal measured 3.3 us per batch).
- Normalization + output projection of batch b deferred until after attention
  of batch b+1 is emitted, hiding the reciprocal chain and output DMA.
- Output partials in bf16 (halves output DMA; host sums partials in f32).
"""
import sys

sys.path.insert(0, "/opt/trn_rl_repo")

import ml_dtypes
import numpy as np

import concourse.bass as bass  # noqa: F401
from concourse import bacc
import concourse.mybir as mybir
import concourse.tile as tile
from concourse.bass_utils import run_bass_kernel_spmd
from concourse.masks import make_identity

B, T, C = 4, 2048, 1024
H, DH = 16, 64
N_CORES = 8
HPC = H // N_CORES          # heads per core = 2
DPC = HPC * DH              # head-dims per core = 128
NT = B * T                  # 8192 tokens
CH = C // 128               # 8 contraction chunks
QB = 512                    # q-block width (moving dim)
KT = 128                    # k-tile width (PE partition dim)
SCALE = 1.0 / 8.0           # 1/sqrt(DH)

F32 = mybir.dt.float32
BF16 = mybir.dt.bfloat16
AF = mybir.ActivationFunctionType
NPBF = ml_dtypes.bfloat16

_CACHED_NC = None
LAST_RESULT = None


def _build():
    nc = bacc.Bacc(None)

    xT = nc.dram_tensor("xT", [C, NT], BF16, kind="ExternalInput")
    wq = nc.dram_tensor("wq", [C, DPC], BF16, kind="ExternalInput")
    wk = nc.dram_tensor("wk", [C, DPC], BF16, kind="ExternalInput")
    wv = nc.dram_tensor("wv", [C, DPC], BF16, kind="ExternalInput")
    bq = nc.dram_tensor("bq", [DPC, 1], F32, kind="ExternalInput")
    bk = nc.dram_tensor("bk", [DPC, 1], F32, kind="ExternalInput")
    bv = nc.dram_tensor("bv", [DPC, 1], F32, kind="ExternalInput")
    wp = nc.dram_tensor("wp", [DPC, C], BF16, kind="ExternalInput")
    emat_in = nc.dram_tensor("emat", [8, 4, 128], BF16, kind="ExternalInput")
    out = nc.dram_tensor("out", [NT, C], BF16, kind="ExternalOutput")

    with tile.TileContext(nc) as tc:
        with (
            tc.tile_pool(name="const", bufs=1) as const,
            tc.tile_pool(name="res", bufs=1) as res,
        ):
            # --- constants ---
            ident = const.tile([128, 128], BF16, tag="ident")
            # sliding causal mask: wmask[k, u] = 1 iff k <= u - 512; a crossing
            # tile r multiplies by wmask[:, 512-128r : 1024-128r]
            wmask = const.tile([128, 1024], BF16, tag="wmask")
            emat = const.tile([8, 4, 128], BF16, tag="emat")
            with tc.tile_pool(name="cstage", bufs=1) as cstage:
                ident_s = cstage.tile([128, 128], F32, tag="ident_s")
                make_identity(nc, ident_s[:])
                nc.vector.tensor_copy(ident[:], ident_s[:])

                wmask_s = cstage.tile([128, 1024], F32, tag="wmask_s")
                nc.gpsimd.memset(wmask_s[:], 0.0)
                nc.gpsimd.affine_select(
                    out=wmask_s[:],
                    in_=wmask_s[:],
                    compare_op=mybir.AluOpType.is_gt,
                    fill=1.0,
                    base=512,
                    # keep 0 where (512 + k - u) > 0, fill 1 where k <= u - 512
                    pattern=[[-1, 1024]],
                    channel_multiplier=1,
                )
                nc.vector.tensor_copy(wmask[:], wmask_s[:])

                nc.sync.dma_start(emat[:], emat_in[:])

            bq_t = const.tile([DPC, 1], F32, tag="bq")
            bk_t = const.tile([DPC, 1], F32, tag="bk")
            bv_t = const.tile([DPC, 1], F32, tag="bv")
            nc.sync.dma_start(bq_t[:], bq[:])
            nc.sync.dma_start(bk_t[:], bk[:])
            nc.sync.dma_start(bv_t[:], bv[:])

            # weights straight into SBUF as bf16 (host pre-converted)
            wq_r = const.tile([128, CH, DPC], BF16, tag="wq_r")
            wk_r = const.tile([128, CH, DPC], BF16, tag="wk_r")
            wv_r = const.tile([128, CH, DPC], BF16, tag="wv_r")
            wp_r = const.tile([DPC, C], BF16, tag="wp_r")
            for w_in, w_dst in ((wq, wq_r), (wk, wk_r), (wv, wv_r)):
                w_re = w_in.rearrange("(c p) n -> p c n", p=128)
                nc.scalar.dma_start(w_dst[:], w_re)
            nc.sync.dma_start(wp_r[:], wp[:])

            # --- residents ---
            qT = res.tile([DPC, NT], BF16, tag="qT")
            kT = res.tile([DPC, NT], BF16, tag="kT")
            vT = res.tile([DPC, NT], BF16, tag="vT")
            yT = res.tile([DPC, NT], BF16, tag="yT")
            # per-batch transposed V (double-buffered so batch b's DMA
            # transposes overlap batch b-1's attention):
            # [tok128, buf, kt, 0:64]=h0 dims, 64=ones, [65:129]=h1, 129=ones
            v_res = res.tile([128, 2, T // KT, 130], BF16, tag="v_res")
            nc.gpsimd.memset(v_res[:, :, :, 64:65], 1.0)
            nc.gpsimd.memset(v_res[:, :, :, 129:130], 1.0)

            # ====== fused pipeline: qkv(b+1) + attention(b) + proj(b-1) ======
            xT_re = xT.rearrange("(c p) t -> c p t", p=128)
            dma_engines = [nc.sync, nc.scalar, nc.gpsimd]
            with (
                tc.tile_pool(name="xpool", bufs=24) as xpool,
                tc.tile_pool(name="epool", bufs=6) as epool,
                tc.tile_pool(name="dpool", bufs=2) as dpool,
                tc.tile_pool(name="opool", bufs=6) as opool,
                tc.tile_pool(name="s_psum", bufs=2, space="PSUM") as s_psum,
                tc.tile_pool(name="y_psum", bufs=2, space="PSUM") as y_psum,
                tc.tile_pool(name="m_psum", bufs=1, space="PSUM") as m_psum,
                tc.tile_pool(name="p_psum", bufs=1, space="PSUM") as p_psum,
            ):
                n_ktiles = T // KT  # 16

                def emit_qkv_tile(tt, head=False):
                    """QKV projection for one 512-token tile: q|k share one
                    wide 2-bank score-pool tile, v rides the py pool. Rider
                    tiles (mid-attention) avoid the scalar DMA queue so the
                    ACT sequencer stays on exp dispatch."""
                    ts_ = slice(tt * QB, (tt + 1) * QB)
                    engines = dma_engines if head else [nc.sync, nc.gpsimd]
                    xrs = []
                    for c in range(CH):
                        xs = xpool.tile([128, QB], BF16, tag="xs")
                        eng = engines[(tt * CH + c) % len(engines)]
                        eng.dma_start(xs[:], xT_re[c, :, ts_])
                        xrs.append(xs)
                    psqk = s_psum.tile([128, 1024], F32, tag="ps", name=f"qk{tt}")
                    psv = y_psum.tile([128, QB], F32, tag="py", name=f"v{tt}")
                    for half, w_r in ((0, wq_r), (1, wk_r)):
                        for c in range(CH):
                            nc.tensor.matmul(
                                psqk[:, half * 512 : (half + 1) * 512],
                                w_r[:, c, :], xrs[c][:],
                                start=(c == 0), stop=(c == CH - 1),
                            )
                    for c in range(CH):
                        nc.tensor.matmul(
                            psv[:], wv_r[:, c, :], xrs[c][:],
                            start=(c == 0), stop=(c == CH - 1),
                        )
                    # copy out of PSUM (+bias; 1/sqrt(dh) is folded into the
                    # host-side Wq, exact since it is a power of two)
                    nc.vector.tensor_scalar_add(qT[:, ts_], psqk[:, :512], bq_t[:])
                    nc.scalar.activation(kT[:, ts_], psqk[:, 512:], AF.Identity, bias=bk_t[:])
                    nc.vector.tensor_scalar_add(vT[:, ts_], psv[:], bv_t[:])

                def emit_v_transpose(b, kt):
                    """PE-transpose one 128-token V tile (both heads at once)
                    for batch b into v_res[:, b%2]."""
                    cb = b * T
                    pt = m_psum.tile([128, 128], BF16, tag="mps", name=f"pt{b}_{kt}")
                    nc.tensor.transpose(
                        pt[:], vT[:, cb + kt * KT : cb + (kt + 1) * KT], ident[:]
                    )
                    nc.vector.tensor_copy(v_res[:, b % 2, kt, 0:64], pt[:, 0:64])
                    nc.vector.tensor_copy(v_res[:, b % 2, kt, 65:129], pt[:, 64:128])

                def emit_norm(b, rec_bf):
                    cb = b * T
                    for qb in range(T // QB):
                        qs = slice(cb + qb * QB, cb + (qb + 1) * QB)
                        pb = s_psum.tile([128, 1024], F32, tag="ps", name=f"pb{b}_{qb}")
                        nc.tensor.matmul(
                            pb[:, :512], emat[:, qb, :], rec_bf[:],
                            start=True, stop=True,
                        )
                        nc.vector.tensor_mul(yT[:, qs], yT[:, qs], pb[:, :512])

                def emit_proj_unit(b, u, alt_pool=False):
                    """One [128 tok, 512 col] half-tile of batch b's partial
                    projection: single matmul + evac + DMA. With alt_pool
                    (the tail, when attention PSUM is idle) rotate across the
                    other pools for depth instead of the single pp slot."""
                    tt, half = u // 2, u % 2
                    trow = b * T + tt * 128
                    if not alt_pool or u % 4 == 0:
                        pp = p_psum.tile([128, 512], F32, tag="pp", name=f"pp{b}_{u}")[:]
                    elif u % 2 == 1:
                        pp = s_psum.tile([128, 1024], F32, tag="ps", name=f"pp{b}_{u}")[:, :512]
                    else:
                        pp = y_psum.tile([128, 512], F32, tag="py", name=f"pp{b}_{u}")[:]
                    nc.tensor.matmul(
                        pp,
                        yT[:, trow : trow + 128],
                        wp_r[:, half * 512 : (half + 1) * 512],
                        start=True, stop=True,
                    )
                    os_ = opool.tile([128, 512], BF16, tag="os", name=f"os{b}_{u}")
                    # evac mostly on DVE: ACT is exp-paced during the
                    # overlapped attention
                    if u % 4 == 1:
                        nc.scalar.copy(os_[:], pp)
                    else:
                        nc.vector.tensor_copy(os_[:], pp)
                    eng = nc.sync if u % 2 == 0 else nc.gpsimd
                    eng.dma_start(
                        out[trow : trow + 128, half * 512 : (half + 1) * 512],
                        os_[:],
                    )

                def emit_attention(b, prev, rec_prev):
                    """Scores+exp+PV for batch b with everything else riding
                    in its pair slots: batch b+1's QKV tiles (pairs 1/11/21/
                    31), prev's norm (after block 0) and 32 projection
                    half-tiles, and batch b+1's V transposes (late pairs).
                    Attention is ACT-exp paced, so the PE slack per pair
                    absorbs the extra matmuls."""
                    cb = b * T
                    denw = dpool.tile([1, 8 * QB], F32, tag="denw", name=f"denw{b}")
                    den = dpool.tile([8, QB], F32, tag="den", name=f"den{b}")
                    # 20 quads per batch, two rider slots each; QKV tiles of
                    # b+1 go in post-PV slots spread across the batch
                    qkv_slots = {1: 0, 6: 1, 11: 2, 16: 3}
                    proj_u = 0
                    vt_kt = 0
                    quad = 0
                    blocks = 0

                    def rider(post_pv):
                        nonlocal proj_u, vt_kt
                        if post_pv and b + 1 < B and quad in qkv_slots:
                            emit_qkv_tile(4 * (b + 1) + qkv_slots[quad])
                        elif blocks >= 1 and prev is not None and proj_u < 32:
                            emit_proj_unit(prev, proj_u)
                            proj_u += 1
                        elif b + 1 < B and vt_kt < n_ktiles and quad >= 16:
                            emit_v_transpose(b + 1, vt_kt)
                            vt_kt += 1

                    # flatten the batch into quad descriptors so scores of
                    # quad i+1 are emitted before PV of quad i (software
                    # pipeline: exp gets a full quad of slack before its PV)
                    quad_descs = []
                    for hl in range(HPC):
                        for qb in range(T // QB):
                            nkt = (qb + 1) * (QB // KT)
                            for q0 in range(nkt // 4):
                                quad_descs.append((hl, qb, q0, nkt))
                    pys = {}

                    def emit_scores(desc):
                        hl, qb, q0, nkt = desc
                        rb = hl * DH
                        qs = slice(cb + qb * QB, cb + (qb + 1) * QB)
                        if q0 == 0:
                            pys[(hl, qb)] = y_psum.tile(
                                [128, QB], F32, tag="py", name=f"py{b}_{hl}_{qb}"
                            )
                        exs = []
                        for p0 in (2 * q0, 2 * q0 + 1):
                            ps2 = s_psum.tile(
                                [128, 1024], F32, tag="ps",
                                name=f"ps{b}_{hl}_{qb}_{p0}",
                            )
                            for j in range(2):
                                kt = 2 * p0 + j
                                nc.tensor.matmul(
                                    ps2[:, j * 512 : (j + 1) * 512],
                                    kT[rb : rb + DH, cb + kt * KT : cb + (kt + 1) * KT],
                                    qT[rb : rb + DH, qs],
                                    start=True, stop=True,
                                )
                            ex2 = epool.tile([128, 1024], BF16, tag="ex", name=f"ex{b}_{hl}_{qb}_{p0}")
                            nc.scalar.activation(ex2[:], ps2[:], AF.Exp)
                            for j in range(2):
                                kt = 2 * p0 + j
                                r = kt - qb * (QB // KT)
                                if r >= 0:
                                    # diagonal-crossing: zero k > q
                                    nc.vector.tensor_mul(
                                        ex2[:, j * 512 : (j + 1) * 512],
                                        ex2[:, j * 512 : (j + 1) * 512],
                                        wmask[:, 512 - r * KT : 1024 - r * KT],
                                    )
                            exs.append(ex2)
                        return exs

                    def emit_pv(desc, exs):
                        nonlocal blocks
                        hl, qb, q0, nkt = desc
                        vlo, vhi = (0, 65) if hl == 0 else (65, 130)
                        py = pys[(hl, qb)]
                        for i in range(4):
                            kt = 4 * q0 + i
                            nc.tensor.matmul(
                                py[:65],
                                v_res[:, b % 2, kt, vlo:vhi],
                                exs[i // 2][:, (i % 2) * 512 : (i % 2 + 1) * 512],
                                start=(kt == 0), stop=(kt == nkt - 1),
                            )
                        if 4 * (q0 + 1) == nkt:
                            # block complete: denominator row + unnormalized y
                            qs = slice(cb + qb * QB, cb + (qb + 1) * QB)
                            p = hl * 4 + qb
                            nc.vector.tensor_copy(
                                denw[:, p * QB : (p + 1) * QB], py[64:65, :]
                            )
                            nc.vector.tensor_copy(yT[hl * DH : hl * DH + DH, qs], py[:DH, :])
                            del pys[(hl, qb)]
                            blocks += 1
                            if blocks == 1 and prev is not None:
                                emit_norm(prev, rec_prev)

                    prev_state = None  # (desc, exs) awaiting PV
                    for desc in quad_descs:
                        exs = emit_scores(desc)
                        if prev_state is not None:
                            rider(post_pv=False)
                            emit_pv(*prev_state)
                            rider(post_pv=True)
                            quad += 1
                        prev_state = (desc, exs)
                    rider(post_pv=False)
                    emit_pv(*prev_state)
                    rider(post_pv=True)
                    # flush any leftover riders
                    while prev is not None and proj_u < 32:
                        emit_proj_unit(prev, proj_u)
                        proj_u += 1
                    while b + 1 < B and vt_kt < n_ktiles:
                        emit_v_transpose(b + 1, vt_kt)
                        vt_kt += 1
                    # scatter den rows to 8 partitions (compute engines can
                    # only write partition bases 0/32/64/96)
                    for p in range(8):
                        nc.sync.dma_start(
                            den[p : p + 1, :], denw[:, p * QB : (p + 1) * QB]
                        )
                    rec = dpool.tile([8, QB], F32, tag="rec", name=f"rec{b}")
                    rec_bf = dpool.tile([8, QB], BF16, tag="rec_bf", name=f"recb{b}")
                    with nc.allow_low_precision(reason="softmax denom recip"):
                        nc.vector.reciprocal_approx_fast(rec[:], den[:])
                        nc.vector.tensor_copy(rec_bf[:], rec[:])
                    return rec_bf

                # head: batch 0's QKV and V transposes stand alone; the rest
                # of the pipeline rides inside attention
                for tt in range(4):
                    emit_qkv_tile(tt, head=True)
                    for kt in range(4 * tt, 4 * tt + 4):
                        emit_v_transpose(0, kt)
                pending = None  # rec_bf of the batch awaiting norm+proj
                for b in range(B):
                    rec_bf = emit_attention(
                        b, b - 1 if pending is not None else None, pending
                    )
                    pending = rec_bf
                emit_norm(B - 1, pending)
                for u in range(32):
                    emit_proj_unit(B - 1, u, alt_pool=True)

    nc.compile()
    return nc


def _get_nc():
    global _CACHED_NC
    if _CACHED_NC is None:
        _CACHED_NC = _build()
    return _CACHED_NC


def kernel(x, W_qkv, b_qkv, W_proj, b_proj, _trace=False, _core_ids=None):
    global LAST_RESULT
    x = np.asarray(x, dtype=np.float32)
    W_qkv = np.asarray(W_qkv, dtype=np.float32)
    b_qkv = np.asarray(b_qkv, dtype=np.float32)
    W_proj = np.asarray(W_proj, dtype=np.float32)
    b_proj = np.asarray(b_proj, dtype=np.float32)

    xT = np.ascontiguousarray(x.reshape(NT, C).T).astype(NPBF)
    emat_np = np.zeros((8, 4, 128), dtype=NPBF)
    for qb in range(4):
        emat_np[qb, qb, :DH] = 1.0
        emat_np[4 + qb, qb, DH:] = 1.0
    core_ids = list(range(N_CORES)) if _core_ids is None else _core_ids
    in_maps = []
    for core in range(len(core_ids)):
        s = slice(core * DPC, (core + 1) * DPC)
        in_maps.append(
            {
                "xT": xT,
                # 1/sqrt(dh) folded into Wq and bq (exact: power of two)
                "wq": (np.ascontiguousarray(W_qkv[:, 0 * C + core * DPC : 0 * C + (core + 1) * DPC]) * np.float32(SCALE)).astype(NPBF),
                "wk": np.ascontiguousarray(W_qkv[:, 1 * C + core * DPC : 1 * C + (core + 1) * DPC]).astype(NPBF),
                "wv": np.ascontiguousarray(W_qkv[:, 2 * C + core * DPC : 2 * C + (core + 1) * DPC]).astype(NPBF),
                "bq": np.ascontiguousarray(b_qkv[0 * C + core * DPC : 0 * C + (core + 1) * DPC, None]) * np.float32(SCALE),
                "bk": np.ascontiguousarray(b_qkv[1 * C + core * DPC : 1 * C + (core + 1) * DPC, None]),
                "bv": np.ascontiguousarray(b_qkv[2 * C + core * DPC : 2 * C + (core + 1) * DPC, None]),
                "wp": np.ascontiguousarray(W_proj[s, :]).astype(NPBF),
                "emat": emat_np,
            }
        )

    nc = _get_nc()
    res = run_bass_kernel_spmd(nc, in_maps, core_ids, trace=_trace)
    LAST_RESULT = res

    acc = np.zeros((NT, C), dtype=np.float32)
    for r in res.results:
        acc += r["out"].astype(np.float32)
    acc += b_proj
    return acc.reshape(B, T, C).astype(np.float32)
